# revision 1
# baseline (speedup 1.0000x reference)
"""Int4-packed linear (group-quantized, 256-group) on 8 Trainium2 cores.

Column-parallel: each core owns 1024 of 8192 out_features.

Math per core (out^T orientation, o on partitions):
  out[t, o] = sum_g s[o,g] * R_g[o,t] - 8*sum_g s[o,g]*xsum_g[t] + bias[o]
  R_g[o,t]  = sum_{i in g} q[o,i] * x[t,i]        (q in 0..15)

Weights ship as fp8e4m3 nibble planes (exact small integers), x as bf16.
Group partials accumulate in PSUM slices; -8 offset + bias ride a tiny fp32
correction matmul into group 31's slice (pre-divided by s[:,31] so the
group-31 scale multiply restores it). Scale multiply = one DVE tensor_tensor
per o-tile with a step-0 broadcast AP; group reduction = GPSIMD tree adds.
"""

import sys

import numpy as np
import ml_dtypes

sys.path.insert(0, "/opt/trn_rl_repo")

import concourse.bass as bass  # noqa: E402
import concourse.mybir as mybir  # noqa: E402
import concourse.tile as tile  # noqa: E402
from concourse import bacc  # noqa: E402

NCORES = 8
TOKENS = 64
IN_F = 8192
OUT_F = 8192
GROUP = 256
OC = OUT_F // NCORES  # 1024 out-features per core
NCHUNK = IN_F // 128  # 64 K-chunks of 128
NG = IN_F // GROUP  # 32 groups
NOT = OC // 128  # 8 o-tiles per core

_cache = {}


def _raw_scan(nc, out, data0, data1, initial, op0, op1):
    """tensor_tensor_scan without the 2D-shape asserts. Multi-free-dim APs
    chain the recurrence across slice boundaries -- intended here: the g=0
    multiplier is 0, cutting the carry at each token boundary."""
    eng = nc.vector
    return eng.add_instruction(
        mybir.InstTensorScalarPtr(
            name=nc.get_next_instruction_name(),
            is_tensor_tensor_scan=True,
            is_scalar_tensor_tensor=True,
            op0=op0,
            op1=op1,
            ins=[
                eng.lower_ap(data0),
                eng.lower_ap_or_imm(initial),
                eng.lower_ap(data1),
            ],
            outs=[eng.lower_ap(out)],
        )
    )


def _build_nc():
    if "nc" in _cache:
        return _cache["nc"], _cache["names"]

    f32 = mybir.dt.float32
    nc = bacc.Bacc(None, target_bir_lowering=False, debug=False)
    with tile.TileContext(nc) as tc:
        with tc.tile_pool(name="dram", bufs=1, space="DRAM") as dram:
            w8 = dram.tile([128, NCHUNK, OC], mybir.dt.float8e4, kind="ExternalInput")
            xt = dram.tile([128, NCHUNK, TOKENS], mybir.dt.bfloat16, kind="ExternalInput")
            sc = dram.tile([128, NOT, NG], f32, kind="ExternalInput")
            # u[p, ot, g] = s[o,g-1]/s[o,g], 0 at g=0 (Horner ratio chain)
            ur = dram.tile([128, NOT, NG], f32, kind="ExternalInput")
            cl = dram.tile([NG + 1, OC], f32, kind="ExternalInput")
            cr = dram.tile([NG + 1, TOKENS], f32, kind="ExternalInput")
            outT = dram.tile([OC, TOKENS], f32, kind="ExternalOutput")

            with (
                tc.tile_pool(name="wsb", bufs=1) as wsb,
                tc.tile_pool(name="xsb", bufs=1) as xsb,
                tc.tile_pool(name="small", bufs=1) as small,
                tc.tile_pool(name="rs", bufs=3) as rspool,
                tc.tile_pool(name="yout", bufs=3) as ypool,
                tc.tile_pool(name="ps", bufs=2, space="PSUM") as ps,
            ):
                w_all = wsb.tile([128, NCHUNK, OC], mybir.dt.float8e4)
                x_all = xsb.tile([128, NCHUNK, TOKENS], mybir.dt.bfloat16)
                sc_all = small.tile([128, NOT, NG], f32)
                u_all = small.tile([128, NOT, NG], f32, tag="u")
                cl_sb = small.tile([NG + 1, OC], f32, tag="cl")
                cr_sb = small.tile([NG + 1, TOKENS], f32, tag="cr")

                # small operands first: every matmul needs x, so it must not
                # queue behind 8.4MB of weights on the HWDGE ring
                nc.sync.dma_start(out=x_all[:], in_=xt[:])
                nc.sync.dma_start(out=sc_all[:], in_=sc[:])
                nc.sync.dma_start(out=u_all[:], in_=ur[:])
                nc.sync.dma_start(out=cl_sb[:], in_=cl[:])
                nc.sync.dma_start(out=cr_sb[:], in_=cr[:])
                # weights in 8 chunk-blocks so matmuls start before all 8.4MB
                for b in range(8):
                    rsl = slice(b * 8, (b + 1) * 8)
                    nc.sync.dma_start(out=w_all[:, rsl, :], in_=w8[:, rsl, :])

                for ot in range(NOT):
                    osl = slice(ot * 128, (ot + 1) * 128)
                    r_ps = ps.tile([128, NG, TOKENS], f32)
                    for g in range(NG):
                        nc.tensor.matmul(
                            r_ps[:, g, :],
                            lhsT=w_all[:, 2 * g, osl],
                            rhs=x_all[:, 2 * g, :],
                            start=True,
                            stop=False,
                        )
                        nc.tensor.matmul(
                            r_ps[:, g, :],
                            lhsT=w_all[:, 2 * g + 1, osl],
                            rhs=x_all[:, 2 * g + 1, :],
                            start=False,
                            stop=(g != NG - 1),
                        )
                    # -8 offset + bias correction, pre-divided by s[:,31]
                    nc.tensor.matmul(
                        r_ps[:, NG - 1, :],
                        lhsT=cl_sb[:, osl],
                        rhs=cr_sb[:],
                        start=False,
                        stop=True,
                    )

                    # fused scale+reduce: Horner scan along g (t outer):
                    #   state(t,g) = u[g]*state + R[g]  with u[0]=0
                    #   => state(t,31) = sum_g R_g * s_g / s_31
                    u_ot = u_all[:, ot, :]
                    u_bcast = bass.AP(
                        tensor=u_ot.tensor,
                        offset=u_ot.offset,
                        ap=[u_ot.ap[0], [0, TOKENS], [1, NG]],
                    )
                    r_tg = bass.AP(
                        tensor=r_ps.tensor,
                        offset=r_ps.offset,
                        ap=[r_ps.ap[0], [1, TOKENS], [TOKENS, NG]],
                    )
                    rs = rspool.tile([128, TOKENS, NG], f32)
                    _raw_scan(
                        nc, rs[:], u_bcast, r_tg, 0.0,
                        mybir.AluOpType.mult, mybir.AluOpType.add,
                    )
                    # y[o, t] = state(t, 31) * s[o, 31]
                    y = ypool.tile([128, TOKENS], f32)
                    nc.vector.tensor_scalar(
                        out=y[:],
                        in0=rs[:, :, NG - 1],
                        scalar1=sc_all[:, ot, NG - 1 : NG],
                        scalar2=None,
                        op0=mybir.AluOpType.mult,
                    )
                    nc.sync.dma_start(out=outT[osl, :], in_=y[:])

    nc.compile()
    names = dict(w8=w8.name, xt=xt.name, sc=sc.name, ur=ur.name, cl=cl.name,
                 cr=cr.name, outT=outT.name)
    _cache["nc"] = nc
    _cache["names"] = names
    return nc, names


def _host_prep(x, weight_packed, scales, bias):
    """Build the 8 per-core input maps."""
    _, names = _build_nc()

    wp = np.ascontiguousarray(weight_packed).view(np.uint32)  # [8192, 1024]
    shifts = (np.arange(8, dtype=np.uint32) * 4)[None, None, :]
    nib = ((wp[:, :, None] >> shifts) & np.uint32(0xF)).astype(np.uint8)
    nib = nib.reshape(OUT_F, IN_F)  # n[o, i]
    lut = np.arange(16, dtype=np.float32).astype(ml_dtypes.float8_e4m3)
    nfp8 = lut[nib]  # [8192, 8192] fp8, exact

    xb = x.astype(ml_dtypes.bfloat16)
    xf = xb.astype(np.float32)
    # xt_host[p, r, t] = x_bf16[t, 128r + p]
    xt_host = np.ascontiguousarray(xb.T.reshape(NCHUNK, 128, TOKENS).transpose(1, 0, 2))
    # xsum_g[t] (with bf16-rounded x, matching the matmul operand)
    xsum = xf.reshape(TOKENS, NG, GROUP).sum(axis=2)  # [t, g]
    cr_host = np.concatenate(
        [xsum.T, np.ones((1, TOKENS), dtype=np.float32)], axis=0
    ).astype(np.float32)  # [33, 64]

    in_maps = []
    for k in range(NCORES):
        osl = slice(OC * k, OC * (k + 1))
        nk = nfp8[osl]  # [1024, 8192]
        # w8_host[p, r, o] = n[o, 128r + p]
        w8_host = np.ascontiguousarray(nk.T.reshape(NCHUNK, 128, OC).transpose(1, 0, 2))
        sck = scales[osl]  # [1024, 32]
        sc_host = np.ascontiguousarray(sck.reshape(NOT, 128, NG).transpose(1, 0, 2))
        uk = np.zeros_like(sck)
        uk[:, 1:] = sck[:, :-1] / sck[:, 1:]
        ur_host = np.ascontiguousarray(uk.reshape(NOT, 128, NG).transpose(1, 0, 2))
        s31 = sck[:, NG - 1]  # [1024]
        cl_host = np.empty((NG + 1, OC), dtype=np.float32)
        cl_host[:NG] = (-8.0 * sck / s31[:, None]).T
        cl_host[NG] = bias[osl] / s31
        in_maps.append({
            names["w8"]: w8_host,
            names["xt"]: xt_host,
            names["sc"]: sc_host.astype(np.float32),
            names["ur"]: ur_host.astype(np.float32),
            names["cl"]: cl_host,
            names["cr"]: cr_host,
        })
    return in_maps


def kernel(x, weight_packed, scales, bias):
    from concourse.bass_utils import run_bass_kernel_spmd

    nc, names = _build_nc()
    in_maps = _host_prep(x, weight_packed, scales, bias)
    res = run_bass_kernel_spmd(nc, in_maps, core_ids=list(range(NCORES)))
    outs = [res.results[k][names["outT"]] for k in range(NCORES)]  # [1024, 64] each
    out = np.concatenate([o.T for o in outs], axis=1)  # [64, 8192]
    return np.ascontiguousarray(out.astype(np.float32))



# revision 2
# speedup vs baseline: 1.3002x; 1.3002x over previous
"""Int4-packed linear (group-quantized, 256-group) on 8 Trainium2 cores.

Column-parallel: each core owns 1024 of 8192 out_features.

Math per core (out^T orientation, o on partitions):
  out[t, o] = sum_g s[o,g] * R_g[o,t] - 8*sum_g s[o,g]*xsum_g[t] + bias[o]
  R_g[o,t]  = sum_{i in g} q[o,i] * x[t,i]        (q in 0..15)

Weights ship as fp8e4m3 nibble planes (exact small integers), x as bf16.
Weight DRAM layout is o-tile-major so each 128-row output tile's full set of
input chunks arrives in one ~1MB DMA and the per-o-tile pipeline
(matmuls -> scale-combine -> store) streams behind the weight DMA.
Group partials accumulate in PSUM slices; -8 offset + bias ride a tiny fp32
correction matmul into group 31's slice (pre-divided by s[:,31] so the
group-31 scale multiply restores it). Scale multiply = one DVE tensor_tensor
per o-tile with a step-0 broadcast AP; group reduction via Horner scan.
"""

import sys

import numpy as np
import ml_dtypes

sys.path.insert(0, "/opt/trn_rl_repo")

import concourse.bass as bass  # noqa: E402
import concourse.mybir as mybir  # noqa: E402
import concourse.tile as tile  # noqa: E402
from concourse import bacc  # noqa: E402

NCORES = 8
TOKENS = 64
IN_F = 8192
OUT_F = 8192
GROUP = 256
OC = OUT_F // NCORES  # 1024 out-features per core
NCHUNK = IN_F // 128  # 64 K-chunks of 128
NG = IN_F // GROUP  # 32 groups
NOT = OC // 128  # 8 o-tiles per core

_cache = {}


def _raw_scan(nc, eng, out, data0, data1, initial, op0, op1):
    """tensor_tensor_scan without the 2D-shape asserts. Multi-free-dim APs
    chain the recurrence across slice boundaries -- intended here: the g=0
    multiplier is 0, cutting the carry at each token boundary."""
    return eng.add_instruction(
        mybir.InstTensorScalarPtr(
            name=nc.get_next_instruction_name(),
            is_tensor_tensor_scan=True,
            is_scalar_tensor_tensor=True,
            op0=op0,
            op1=op1,
            ins=[
                eng.lower_ap(data0),
                eng.lower_ap_or_imm(initial),
                eng.lower_ap(data1),
            ],
            outs=[eng.lower_ap(out)],
        )
    )


def _build_nc():
    if "nc" in _cache:
        return _cache["nc"], _cache["names"]

    f32 = mybir.dt.float32
    nc = bacc.Bacc(None, target_bir_lowering=False, debug=False)
    with tile.TileContext(nc) as tc:
        with tc.tile_pool(name="dram", bufs=1, space="DRAM") as dram:
            # w8[p, ot, r, c] = nibble_fp8[ot*128 + c, 128*r + p]
            w8 = dram.tile([128, NOT, NCHUNK, 128], mybir.dt.float8e4,
                           kind="ExternalInput")
            xt = dram.tile([128, NCHUNK, TOKENS], mybir.dt.bfloat16, kind="ExternalInput")
            sc = dram.tile([128, NOT, NG], f32, kind="ExternalInput")
            # u[p, ot, g] = s[o,g-1]/s[o,g], 0 at g=0 (Horner ratio chain)
            ur = dram.tile([128, NOT, NG], f32, kind="ExternalInput")
            cl = dram.tile([NG + 1, OC], f32, kind="ExternalInput")
            cr = dram.tile([NG + 1, TOKENS], f32, kind="ExternalInput")
            outT = dram.tile([OC, TOKENS], f32, kind="ExternalOutput")

            with (
                tc.tile_pool(name="wsb", bufs=1) as wsb,
                tc.tile_pool(name="xsb", bufs=1) as xsb,
                tc.tile_pool(name="small", bufs=1) as small,
                tc.tile_pool(name="rs", bufs=3) as rspool,
                tc.tile_pool(name="yout", bufs=3) as ypool,
                tc.tile_pool(name="ps", bufs=2, space="PSUM") as ps,
            ):
                w_all = wsb.tile([128, NOT, NCHUNK, 128], mybir.dt.float8e4)
                x_all = xsb.tile([128, NCHUNK, TOKENS], mybir.dt.bfloat16)
                sc_all = small.tile([128, NOT, NG], f32)
                u_all = small.tile([128, NOT, NG], f32, tag="u")
                cl_sb = small.tile([NG + 1, OC], f32, tag="cl")
                cr_sb = small.tile([NG + 1, TOKENS], f32, tag="cr")

                # tiny operands ride the Scalar HWDGE queue so the Sync queue
                # streams x + weights back-to-back in consumption order
                nc.scalar.dma_start(out=sc_all[:], in_=sc[:])
                nc.scalar.dma_start(out=u_all[:], in_=ur[:])
                nc.scalar.dma_start(out=cl_sb[:], in_=cl[:])
                nc.scalar.dma_start(out=cr_sb[:], in_=cr[:])

                # consumption-ordered weight stream: x first half, o-tile 0
                # (halved for a faster ramp), x second half, o-tiles 1-7
                nc.sync.dma_start(out=x_all[:, 0:32, :], in_=xt[:, 0:32, :])
                nc.sync.dma_start(out=w_all[:, 0, 0:32, :], in_=w8[:, 0, 0:32, :])
                nc.sync.dma_start(out=x_all[:, 32:64, :], in_=xt[:, 32:64, :])
                nc.sync.dma_start(out=w_all[:, 0, 32:64, :], in_=w8[:, 0, 32:64, :])
                for ot in range(1, NOT):
                    nc.sync.dma_start(out=w_all[:, ot, :, :], in_=w8[:, ot, :, :])

                for ot in range(NOT):
                    osl = slice(ot * 128, (ot + 1) * 128)
                    r_ps = ps.tile([128, NG, TOKENS], f32)
                    for g in range(NG):
                        nc.tensor.matmul(
                            r_ps[:, g, :],
                            lhsT=w_all[:, ot, 2 * g, :],
                            rhs=x_all[:, 2 * g, :],
                            start=True,
                            stop=False,
                        )
                        nc.tensor.matmul(
                            r_ps[:, g, :],
                            lhsT=w_all[:, ot, 2 * g + 1, :],
                            rhs=x_all[:, 2 * g + 1, :],
                            start=False,
                            stop=(g != NG - 1),
                        )
                    # -8 offset + bias correction, pre-divided by s[:,31]
                    nc.tensor.matmul(
                        r_ps[:, NG - 1, :],
                        lhsT=cl_sb[:, osl],
                        rhs=cr_sb[:],
                        start=False,
                        stop=True,
                    )

                    # fused scale+reduce: Horner scan along g (t outer):
                    #   state(t,g) = u[g]*state + R[g]  with u[0]=0
                    #   => state(t,31) = sum_g R_g * s_g / s_31
                    u_ot = u_all[:, ot, :]
                    u_bcast = bass.AP(
                        tensor=u_ot.tensor,
                        offset=u_ot.offset,
                        ap=[u_ot.ap[0], [0, TOKENS], [1, NG]],
                    )
                    r_tg = bass.AP(
                        tensor=r_ps.tensor,
                        offset=r_ps.offset,
                        ap=[r_ps.ap[0], [1, TOKENS], [TOKENS, NG]],
                    )
                    rs = rspool.tile([128, TOKENS, NG], f32)
                    _raw_scan(
                        nc, nc.vector, rs[:], u_bcast, r_tg, 0.0,
                        mybir.AluOpType.mult, mybir.AluOpType.add,
                    )
                    # y[o, t] = state(t, 31) * s[o, 31]
                    y = ypool.tile([128, TOKENS], f32)
                    nc.vector.tensor_scalar(
                        out=y[:],
                        in0=rs[:, :, NG - 1],
                        scalar1=sc_all[:, ot, NG - 1 : NG],
                        scalar2=None,
                        op0=mybir.AluOpType.mult,
                    )
                    nc.scalar.dma_start(out=outT[osl, :], in_=y[:])

    nc.compile()
    names = dict(w8=w8.name, xt=xt.name, sc=sc.name, ur=ur.name, cl=cl.name,
                 cr=cr.name, outT=outT.name)
    _cache["nc"] = nc
    _cache["names"] = names
    return nc, names


def _host_prep(x, weight_packed, scales, bias):
    """Build the 8 per-core input maps."""
    _, names = _build_nc()

    wp = np.ascontiguousarray(weight_packed).view(np.uint32)  # [8192, 1024]
    shifts = (np.arange(8, dtype=np.uint32) * 4)[None, None, :]
    nib = ((wp[:, :, None] >> shifts) & np.uint32(0xF)).astype(np.uint8)
    nib = nib.reshape(OUT_F, IN_F)  # n[o, i]
    lut = np.arange(16, dtype=np.float32).astype(ml_dtypes.float8_e4m3)
    nfp8 = lut[nib]  # [8192, 8192] fp8, exact

    xb = x.astype(ml_dtypes.bfloat16)
    xf = xb.astype(np.float32)
    # xt_host[p, r, t] = x_bf16[t, 128r + p]
    xt_host = np.ascontiguousarray(xb.T.reshape(NCHUNK, 128, TOKENS).transpose(1, 0, 2))
    # xsum_g[t] (with bf16-rounded x, matching the matmul operand)
    xsum = xf.reshape(TOKENS, NG, GROUP).sum(axis=2)  # [t, g]
    cr_host = np.concatenate(
        [xsum.T, np.ones((1, TOKENS), dtype=np.float32)], axis=0
    ).astype(np.float32)  # [33, 64]

    in_maps = []
    for k in range(NCORES):
        osl = slice(OC * k, OC * (k + 1))
        nk = nfp8[osl]  # [1024, 8192]
        # w8_host[p, ot, r, c] = nk[ot*128 + c, 128*r + p]
        w8_host = np.ascontiguousarray(
            nk.reshape(NOT, 128, NCHUNK, 128).transpose(3, 0, 2, 1)
        )
        sck = scales[osl]  # [1024, 32]
        sc_host = np.ascontiguousarray(sck.reshape(NOT, 128, NG).transpose(1, 0, 2))
        uk = np.zeros_like(sck)
        uk[:, 1:] = sck[:, :-1] / sck[:, 1:]
        ur_host = np.ascontiguousarray(uk.reshape(NOT, 128, NG).transpose(1, 0, 2))
        s31 = sck[:, NG - 1]  # [1024]
        cl_host = np.empty((NG + 1, OC), dtype=np.float32)
        cl_host[:NG] = (-8.0 * sck / s31[:, None]).T
        cl_host[NG] = bias[osl] / s31
        in_maps.append({
            names["w8"]: w8_host,
            names["xt"]: xt_host,
            names["sc"]: sc_host.astype(np.float32),
            names["ur"]: ur_host.astype(np.float32),
            names["cl"]: cl_host,
            names["cr"]: cr_host,
        })
    return in_maps


def kernel(x, weight_packed, scales, bias):
    from concourse.bass_utils import run_bass_kernel_spmd

    nc, names = _build_nc()
    in_maps = _host_prep(x, weight_packed, scales, bias)
    res = run_bass_kernel_spmd(nc, in_maps, core_ids=list(range(NCORES)))
    outs = [res.results[k][names["outT"]] for k in range(NCORES)]  # [1024, 64] each
    out = np.concatenate([o.T for o in outs], axis=1)  # [64, 8192]
    return np.ascontiguousarray(out.astype(np.float32))
